# revision 1
# baseline (speedup 1.0000x reference)
"""GQA prefill attention (B=2, S=2048, D=2048, H=32, KV=8, HD=64) on 8 trn2 cores.

Sharding: tensor-parallel over heads. Core c owns q-heads [4c, 4c+4) and
kv-head c (n_rep=4), computes its partial of out = attn_out @ wo; host sums
the 8 partials.

Device kernel (per core, bf16 matmuls / fp32 PSUM):
  QT[dh,s] = wq_c^T-chunks @ xT      (lhsT=wq chunk, rhs=xT chunk)
  KT[dh,s], VT[dh,s] packed in one stream; VT transposed back to V[k,dh] on PE
  RoPE via pair-swap permutation matmul + elementwise cos/sin tables
  ST[k,q] = KT-chunk^T @ QT          (scores transposed; softmax w/o max-sub)
  P = exp(ST/8) (* mask-multiplier tile for partially-masked blocks)
  O[q,65] += P-chunk^T @ [V | 1]     (ones column gives the softmax row-sum)
  O normalized by 1/rowsum, transposed to OT[dh,q] on PE
  out_partial[s,:] = OT-chunks^T @ wo_c
"""

import os
import sys

import numpy as np
import ml_dtypes

BF16 = ml_dtypes.bfloat16

B, S, D, H, KV, HD = 2, 2048, 2048, 32, 8, 64
NCORES = 8
HPC = H // NCORES  # 4 q-heads per core
QS_TILES = S // 512  # 4 q-stripes of 512 per batch
KT_TILES = S // 128  # 16 k-blocks of 128


def _host_prepare(x, wq, wk, wv, wo, freqs, mask):
    """Build per-core device inputs + the mask block schedule."""
    # xT[b, d, s] bf16
    xT = np.ascontiguousarray(x.transpose(0, 2, 1)).astype(BF16)

    # RoPE tables in the [dh-on-partitions, s] layout used by QT/KT.
    # Two 64-row head copies stacked (head pairs live on 128 partitions).
    # rope: out[2j]   = t[2j] cos - t[2j+1] sin
    #       out[2j+1] = t[2j] sin + t[2j+1] cos
    # with swap(t)[d] = t[d^1]:  out[d] = t[d]*cos[d] + swap(t)[d]*sgn(d)*sin[d]
    c64 = np.cos(freqs.T).repeat(2, axis=0).astype(np.float64)  # [64, S]
    s64 = np.sin(freqs.T).repeat(2, axis=0).astype(np.float64)
    sgn = np.where(np.arange(HD) % 2 == 0, -1.0, 1.0)[:, None]
    cos_t = np.concatenate([c64, c64], axis=0).astype(BF16)           # [128, S]
    sin_t = np.concatenate([s64 * sgn, s64 * sgn], axis=0).astype(BF16)

    # Mask block schedule at [128 k x 512 q] granularity (same for all b, h).
    # Block (qs, kt): full (mask all zero), skip (all <= -30), or masked
    # (multiply exp'd P by exp(mask^T) tile).
    mt_tiles = []  # unique [128, 512] multiplier tiles
    mt_keys = {}
    sched = []  # per qs: list of (kt, mtile_idx | None, jlo)
    for qs in range(QS_TILES):
        lst = []
        for kt in range(KT_TILES):
            blk = mask[qs * 512:(qs + 1) * 512, kt * 128:(kt + 1) * 128]  # [q, k]
            if np.all(blk <= -30.0):
                continue
            # first 128-q subblock with any visible entry; only trust a
            # clean fully-masked prefix, else compute the whole stripe
            jmasked = [np.all(blk[j * 128:(j + 1) * 128] <= -30.0) for j in range(4)]
            jlo = 0
            while jlo < 4 and jmasked[jlo]:
                jlo += 1
            if any(jmasked[jlo:]):
                jlo = 0
            vis = blk[jlo * 128:]
            if np.all(vis == 0.0):
                lst.append((kt, None, jlo))
                continue
            tile_np = np.exp(blk.T.astype(np.float64)).astype(BF16)  # [128k, 512q]
            key = tile_np.tobytes()
            if key not in mt_keys:
                mt_keys[key] = len(mt_tiles)
                mt_tiles.append(tile_np)
            lst.append((kt, mt_keys[key], jlo))
        # if some q-subblock has no contributing kt at all, fall back to
        # full-width compute so its softmax denominator stays well-defined
        for j in range(4):
            if not any(e[2] <= j for e in lst):
                lst = [(kt, mi, 0) for (kt, mi, _) in lst]
                break
        sched.append(lst)
    if not mt_tiles:  # keep the input well-formed even if no masked blocks
        mt_tiles.append(np.ones((128, 512), dtype=BF16))
    mt = np.stack(mt_tiles)  # [U, 128, 512]

    per_core = []
    for c in range(NCORES):
        per_core.append({
            "xT": xT,
            "wq": np.ascontiguousarray(wq[:, c * HPC * HD:(c + 1) * HPC * HD]).astype(BF16),
            "wkv": np.ascontiguousarray(
                np.concatenate([wk[:, c * HD:(c + 1) * HD], wv[:, c * HD:(c + 1) * HD]], axis=1)
            ).astype(BF16),
            "wo": np.ascontiguousarray(wo[c * HPC * HD:(c + 1) * HPC * HD, :]).astype(BF16),
            "cos": cos_t,
            "sin": sin_t,
            "mt": mt,
        })
    return per_core, sched, mt.shape[0]


def _build_program(sched, U):
    import concourse.bass as bass
    import concourse.mybir as mybir
    import concourse.tile as tile
    from concourse import bacc

    dt = mybir.dt
    bf, f32 = dt.bfloat16, dt.float32
    AF = mybir.ActivationFunctionType

    nc = bacc.Bacc("TRN2", target_bir_lowering=False, debug=False,
                   num_devices=NCORES)

    xT = nc.dram_tensor("xT", [B, D, S], bf, kind="ExternalInput")
    wq = nc.dram_tensor("wq", [D, HPC * HD], bf, kind="ExternalInput")
    wkv = nc.dram_tensor("wkv", [D, 2 * HD], bf, kind="ExternalInput")
    wo = nc.dram_tensor("wo", [HPC * HD, D], bf, kind="ExternalInput")
    cos = nc.dram_tensor("cos", [128, S], bf, kind="ExternalInput")
    sin = nc.dram_tensor("sin", [128, S], bf, kind="ExternalInput")
    mt = nc.dram_tensor("mt", [U, 128, 512], bf, kind="ExternalInput")
    out = nc.dram_tensor("out", [B, S, D], f32, kind="ExternalOutput")

    # pair-swap permutation (block-diag over the two stacked 64-row heads)
    perm_np = np.zeros((128, 128), dtype=BF16)
    for d in range(128):
        perm_np[d ^ 1, d] = 1
    perm_dram = nc.inline_tensor(np.ascontiguousarray(perm_np), name="perm")
    ident_dram = nc.inline_tensor(np.eye(128, dtype=BF16), name="ident")

    DC = D // 128  # 16 contraction chunks

    with tile.TileContext(nc) as tc:
        with tc.tile_pool(name="const", bufs=1) as cp:
            wq_sb = cp.tile([128, DC, HPC * HD], bf)
            nc.sync.dma_start(wq_sb[:], wq.ap().rearrange("(c p) m -> p c m", p=128))
            wkv_sb = cp.tile([128, DC, 2 * HD], bf)
            nc.sync.dma_start(wkv_sb[:], wkv.ap().rearrange("(c p) m -> p c m", p=128))
            wo_sb = cp.tile([128, 2, D], bf)
            nc.sync.dma_start(wo_sb[:], wo.ap().rearrange("(g p) n -> p g n", p=128))
            cos_sb = cp.tile([128, S], bf)
            nc.sync.dma_start(cos_sb[:], cos.ap())
            sin_sb = cp.tile([128, S], bf)
            nc.sync.dma_start(sin_sb[:], sin.ap())
            mt_sb = cp.tile([128, U, 512], bf)
            nc.sync.dma_start(mt_sb[:], mt.ap().rearrange("u p q -> p u q"))
            perm_sb = cp.tile([128, 128], bf)
            nc.sync.dma_start(perm_sb[:], perm_dram.ap())
            ident_sb = cp.tile([128, 128], bf)
            nc.sync.dma_start(ident_sb[:], ident_dram.ap())

            qt_sb = cp.tile([64, B, HPC, S], bf)   # [dh, b, head, s] (base-0)
            kt_sb = cp.tile([64, B, S], bf)        # [dh, b, s] (base-0)
            vone_sb = cp.tile([128, B, KT_TILES, HD + 1], bf)  # [k%128, b, kt, dh|1]
            nc.vector.memset(vone_sb[:, :, :, HD:HD + 1], 1.0)

            # ---------------- phase 1: projections + rope ----------------
            with (
                tc.tile_pool(name="xt", bufs=2) as xp,
                tc.tile_pool(name="raw", bufs=2) as rawp,
                tc.tile_pool(name="rtmp", bufs=2) as rtp,
                tc.tile_pool(name="ps_q0", bufs=2, space="PSUM") as pq0,
                tc.tile_pool(name="ps_q1", bufs=2, space="PSUM") as pq1,
                tc.tile_pool(name="ps_kv", bufs=2, space="PSUM") as pkv,
                tc.tile_pool(name="ps_sw", bufs=1, space="PSUM") as psw,
                tc.tile_pool(name="ps_vt", bufs=1, space="PSUM") as pvt,
            ):
                for b in range(B):
                    for st in range(S // 512):
                        s0 = st * 512
                        xbig = xp.tile([128, DC, 512], bf)
                        nc.sync.dma_start(
                            xbig[:],
                            xT.ap()[b].rearrange("(c p) s -> p c s", p=128)[:, :, s0:s0 + 512],
                        )
                        q0p = pq0.tile([128, 512], f32)
                        q1p = pq1.tile([128, 512], f32)
                        kvp = pkv.tile([128, 512], f32)
                        for dc in range(DC):
                            nc.tensor.matmul(q0p[:], lhsT=wq_sb[:, dc, 0:128],
                                             rhs=xbig[:, dc, :],
                                             start=(dc == 0), stop=(dc == DC - 1))
                            nc.tensor.matmul(q1p[:], lhsT=wq_sb[:, dc, 128:256],
                                             rhs=xbig[:, dc, :],
                                             start=(dc == 0), stop=(dc == DC - 1))
                            nc.tensor.matmul(kvp[:], lhsT=wkv_sb[:, dc, :],
                                             rhs=xbig[:, dc, :],
                                             start=(dc == 0), stop=(dc == DC - 1))
                        # raw copies to SBUF (also the swap-matmul inputs)
                        q0r = rawp.tile([128, 512], bf, tag="q0r")
                        nc.scalar.copy(q0r[:], q0p[:])
                        q1r = rawp.tile([128, 512], bf, tag="q1r")
                        nc.scalar.copy(q1r[:], q1p[:])
                        kvr = rawp.tile([128, 512], bf, tag="kvr")
                        nc.scalar.copy(kvr[:], kvp[:])
                        # V: transpose VT rows back to natural [k, dh]
                        for j in range(4):
                            vtp = pvt.tile([128, HD], bf)
                            nc.tensor.transpose(vtp[:], kvr[64:128, j * 128:(j + 1) * 128],
                                                ident_sb[64:128, 64:128])
                            nc.vector.tensor_copy(vone_sb[:, b, 4 * st + j, 0:HD], vtp[:])
                        # rope Q (both pairs)
                        for pb, qr in ((0, q0r), (1, q1r)):
                            swp = psw.tile([128, 512], f32, tag="sw")
                            nc.tensor.matmul(swp[:], lhsT=perm_sb[:], rhs=qr[:],
                                             start=True, stop=True)
                            t_sin = rtp.tile([128, 512], bf, tag="tsin")
                            nc.vector.tensor_mul(t_sin[:], swp[:], sin_sb[:, s0:s0 + 512])
                            t_cos = rtp.tile([128, 512], bf, tag="tcos")
                            nc.vector.tensor_mul(t_cos[:], qr[:], cos_sb[:, s0:s0 + 512])
                            nc.vector.tensor_add(qt_sb[:, b, 2 * pb, s0:s0 + 512],
                                                 t_sin[0:64, :], t_cos[0:64, :])
                            nc.vector.tensor_add(qt_sb[:, b, 2 * pb + 1, s0:s0 + 512],
                                                 t_sin[64:128, :], t_cos[64:128, :])
                        # rope K (rows 0:64 of kv)
                        ksw = psw.tile([64, 512], f32, tag="sw")
                        nc.tensor.matmul(ksw[:], lhsT=perm_sb[0:64, 0:64],
                                         rhs=kvr[0:64, :], start=True, stop=True)
                        k_sin = rtp.tile([64, 512], bf, tag="tsin")
                        nc.vector.tensor_mul(k_sin[:], ksw[:], sin_sb[0:64, s0:s0 + 512])
                        k_cos = rtp.tile([64, 512], bf, tag="tcos")
                        nc.vector.tensor_mul(k_cos[:], kvr[0:64, :], cos_sb[0:64, s0:s0 + 512])
                        nc.vector.tensor_add(kt_sb[:, b, s0:s0 + 512],
                                             k_sin[:], k_cos[:])

            # ---------------- phase 2: attention + wo ----------------
            with (
                tc.tile_pool(name="pp", bufs=3) as ppool,
                tc.tile_pool(name="osb", bufs=3) as osp,
                tc.tile_pool(name="rcp", bufs=3) as rcp,
                tc.tile_pool(name="ot", bufs=2) as otp,
                tc.tile_pool(name="wsb", bufs=3) as wsp,
                tc.tile_pool(name="ps_s", bufs=2, space="PSUM") as pss,
                tc.tile_pool(name="ps_o", bufs=1, space="PSUM") as pso,
                tc.tile_pool(name="ps_t", bufs=1, space="PSUM") as pst,
                tc.tile_pool(name="ps_w", bufs=1, space="PSUM") as psw2,
            ):
                for b in range(B):
                    for qs in range(QS_TILES):
                        q0 = qs * 512
                        ot_t = otp.tile([128, 2, 512], bf)  # [dh-in-pair, pair, q]
                        kts = sched[qs]
                        first_kt = [min(e[0] for e in kts if e[2] <= j)
                                    for j in range(4)]
                        last_kt = [max(e[0] for e in kts if e[2] <= j)
                                   for j in range(4)]
                        for h in range(HPC):
                            pb, po = h // 2, (h % 2) * 64
                            ops = [pso.tile([128, HD + 1], f32, tag=f"opsum{_j}",
                                            name=f"opsum{_j}")
                                   for _j in range(4)]
                            for kt, mi, jlo in kts:
                                nq = 512 - jlo * 128
                                sp = pss.tile([128, 512], f32)
                                nc.tensor.matmul(
                                    sp[:, 0:nq],
                                    lhsT=kt_sb[:, b, kt * 128:(kt + 1) * 128],
                                    rhs=qt_sb[:, b, h, q0 + jlo * 128:q0 + 512],
                                    start=True, stop=True)
                                pt = ppool.tile([128, 512], bf)
                                nc.scalar.activation(pt[:, 0:nq], sp[:, 0:nq], AF.Exp,
                                                     scale=1.0 / np.sqrt(HD))
                                if mi is not None:
                                    nc.vector.tensor_mul(pt[:, 0:nq], pt[:, 0:nq],
                                                         mt_sb[:, mi, jlo * 128:512])
                                for j in range(jlo, 4):
                                    nc.tensor.matmul(
                                        ops[j][:],
                                        lhsT=pt[:, (j - jlo) * 128:(j - jlo + 1) * 128],
                                        rhs=vone_sb[:, b, kt, :],
                                        start=(kt == first_kt[j]),
                                        stop=(kt == last_kt[j]))
                            for j in range(4):
                                rc = rcp.tile([128, 1], f32)
                                nc.vector.reciprocal(rc[:], ops[j][:, HD:HD + 1])
                                osb = osp.tile([128, HD], bf)
                                nc.vector.tensor_scalar_mul(osb[:], ops[j][:, 0:HD], rc[:])
                                tp = pst.tile([64, 128], bf)
                                nc.tensor.transpose(tp[:], osb[:], ident_sb[:])
                                nc.vector.tensor_copy(
                                    ot_t[po:po + 64, pb, j * 128:(j + 1) * 128], tp[:])
                        # wo for this (b, q-stripe)
                        for j in range(4):
                            sq0 = q0 + j * 128
                            for nb in range(4):
                                wp = psw2.tile([128, 512], f32)
                                nc.tensor.matmul(wp[:], lhsT=ot_t[:, 0, j * 128:(j + 1) * 128],
                                                 rhs=wo_sb[:, 0, nb * 512:(nb + 1) * 512],
                                                 start=True, stop=False)
                                nc.tensor.matmul(wp[:], lhsT=ot_t[:, 1, j * 128:(j + 1) * 128],
                                                 rhs=wo_sb[:, 1, nb * 512:(nb + 1) * 512],
                                                 start=False, stop=True)
                                wsb = wsp.tile([128, 512], f32)
                                nc.vector.tensor_copy(wsb[:], wp[:])
                                nc.sync.dma_start(
                                    out.ap()[b, sq0:sq0 + 128, nb * 512:(nb + 1) * 512],
                                    wsb[:])
    nc.compile()
    return nc


def kernel(x, wq, wk, wv, wo, freqs, mask, start_pos):
    sys.path.insert(0, "/opt/trn_rl_repo")
    from concourse.bass_utils import run_bass_kernel_spmd

    x = np.asarray(x, dtype=np.float32)
    per_core, sched, U = _host_prepare(
        x, np.asarray(wq, np.float32), np.asarray(wk, np.float32),
        np.asarray(wv, np.float32), np.asarray(wo, np.float32),
        np.asarray(freqs, np.float32), np.asarray(mask, np.float32))

    nc = _build_program(sched, U)

    trace = bool(int(os.environ.get("BASSKERNEL_TRACE", "0")))
    if trace and "antenv.axon_hooks" not in sys.modules:
        # profile-hook shim (the trimmed antenv package lacks axon_hooks)
        try:
            import types

            if "/root/.axon_site" not in sys.path:
                sys.path.insert(0, "/root/.axon_site")
            from trn_agent_boot.trn_boot import _ntff_profile_via_ctypes

            _hook = _ntff_profile_via_ctypes("/opt/axon/libaxon_pjrt.so")
            _mod = types.ModuleType("antenv.axon_hooks")
            _mod.get_axon_ntff_profile_hook = lambda: _hook
            _mod.set_axon_ntff_profile_hook = lambda h: None
            sys.modules["antenv.axon_hooks"] = _mod
        except Exception:
            trace = False
    res = run_bass_kernel_spmd(nc, per_core, core_ids=list(range(NCORES)),
                               trace=trace)
    if trace:
        kernel._last_exec_time_ns = res.exec_time_ns
        kernel._last_profile = res.profile_json
    acc = res.results[0]["out"].astype(np.float64)
    for c in range(1, NCORES):
        acc += res.results[c]["out"].astype(np.float64)
    return acc.astype(np.float32)

